# revision 1
# baseline (speedup 1.0000x reference)
"""Trainium2 Bass kernel for nn_ColorFeatureExtractor (per-image KMeans color
extraction). Pure data parallel: image b -> core b. Each core runs 100 Lloyd
iterations entirely on-chip and streams back per-iteration counts + centers
trajectories; the host selects the convergence iteration (faithful to the
reference's global-allclose freeze semantics) and assembles the [B,K,K,4]
output."""
import sys
import numpy as np

for _p in ("/opt/trn_rl_repo", "/root/.axon_site/_ro/trn_rl_repo"):
    if _p not in sys.path:
        sys.path.append(_p)

K = 5
N = 224 * 224          # pixels per image
P = 128                # partitions
F = N // P             # 392 free elems per partition
import os
ITERS = int(os.environ.get("KM_ITERS", "100"))
RTOL, ATOL = 1e-5, 1e-8
OUT_LEN = 500 + 101 * 15   # counts traj + centers traj

_CACHE = {}


def _build_nc():
    import concourse.bass as bass
    import concourse.mybir as mybir
    from concourse import bacc, tile

    f32 = mybir.dt.float32
    Alu = mybir.AluOpType
    Act = mybir.ActivationFunctionType

    nc = bacc.Bacc(None, target_bir_lowering=False)
    xp = nc.dram_tensor("xp", [3, N], f32, kind="ExternalInput")
    cbin = nc.dram_tensor("cbin", [1, 20], f32, kind="ExternalInput")
    outv = nc.dram_tensor("outv", [1, OUT_LEN], f32, kind="ExternalOutput")

    with tile.TileContext(nc) as tc:
        with (
            tc.tile_pool(name="persist", bufs=1) as pp,
            tc.tile_pool(name="sbig", bufs=2) as sb,
            tc.tile_pool(name="scr", bufs=3) as scr,
            tc.tile_pool(name="small", bufs=2) as sm,
            tc.tile_pool(name="psum", bufs=2, space=bass.MemorySpace.PSUM) as ps,
        ):
            # ---- persistent tiles ----
            px = pp.tile([P, F], f32, tag="px")
            py = pp.tile([P, F], f32, tag="py")
            pz = pp.tile([P, F], f32, tag="pz")
            ones_col = pp.tile([P, 1], f32, tag="ones_col")    # matmul lhsT for col-sum
            ones_row = pp.tile([1, P], f32, tag="ones_row")    # matmul lhsT for broadcast
            tot3 = pp.tile([1, 3], f32, tag="tot3")            # sum of px/py/pz
            counts_st = pp.tile([1, 500], f32, tag="counts_st")
            cent_st = pp.tile([1, 101 * 15], f32, tag="cent_st")

            nc.vector.memset(counts_st[:], 0.0)
            nc.vector.memset(cent_st[:], 0.0)
            xap = xp[:].rearrange("c (p f) -> c p f", p=P)
            nc.sync.dma_start(out=px[:], in_=xap[0])
            nc.sync.dma_start(out=py[:], in_=xap[1])
            nc.sync.dma_start(out=pz[:], in_=xap[2])
            cb0 = pp.tile([1, 20], f32, tag="cb0")
            nc.sync.dma_start(out=cb0[:], in_=cbin[:])

            nc.vector.memset(ones_col[:], 1.0)
            nc.vector.memset(ones_row[:], 1.0)

            # pixels = x + 1e-8, vector-owned; gpsimd gets private copies so
            # its loop-body ops never need cross-engine waits (HW structs have
            # very few sync-wait slots)
            nc.vector.tensor_scalar(px[:], px[:], 1e-8, None, Alu.add)
            nc.vector.tensor_scalar(py[:], py[:], 1e-8, None, Alu.add)
            nc.vector.tensor_scalar(pz[:], pz[:], 1e-8, None, Alu.add)


            planes0 = (px, py, pz)
            # totals: [1,3] = sum of each plane
            totc = pp.tile([P, 3], f32, tag="totc")
            nc.vector.tensor_reduce(totc[:, 0:1], px[:], mybir.AxisListType.X, Alu.add)
            nc.vector.tensor_reduce(totc[:, 1:2], py[:], mybir.AxisListType.X, Alu.add)
            nc.vector.tensor_reduce(totc[:, 2:3], pz[:], mybir.AxisListType.X, Alu.add)
            tot3_ps = ps.tile([1, 3], f32, tag="tot3ps")
            nc.tensor.matmul(tot3_ps[:], ones_col[:], totc[:], start=True, stop=True)
            nc.vector.tensor_copy(tot3[:], tot3_ps[:])

            # interleaved pixel tile [p, f*3] = (x,y,z) per pixel, for the
            # one-TT-per-cluster product in phase 3
            pint = pp.tile([P, 3 * F], f32, tag="pint")
            for d in range(3):
                nc.vector.tensor_copy(
                    pint[:].rearrange("p (f d) -> p d f", d=3)[:, d], planes0[d][:]
                )

            # initial centers into trajectory + initial rep broadcast
            nc.scalar.copy(cent_st[0:1, 0:15], cb0[0:1, 0:15])

            cb0v = pp.tile([1, 20], f32, tag="cb0v")
            nc.vector.tensor_copy(cb0v[:], cb0[:])
            rep_ps0 = ps.tile([P, 20], f32, tag="repps")
            nc.tensor.matmul(rep_ps0[:], ones_row[:], cb0v[:], start=True, stop=True)
            rep = sb.tile([P, 20], f32, tag="rep")
            nc.vector.tensor_copy(rep[:], rep_ps0[:])

            for t in range(1, ITERS + 1):
                # ---------- phase 1: scores s_k = px*cx + py*cy + pz*cz + b ----------
                s_tiles = []
                for k in range(5):
                    u = scr.tile([P, F], f32, tag=f"u{k}")
                    # u = px*cx_k + b_k (ACT free affine with AP scale/bias)
                    nc.scalar.activation(
                        u[:], px[:], Act.Identity,
                        bias=rep[:, 15 + k : 16 + k], scale=rep[:, 3 * k : 3 * k + 1],
                    )
                    v = scr.tile([P, F], f32, tag=f"v{k}")
                    s = sb.tile([P, F], f32, tag=f"s{k}")
                    nc.vector.scalar_tensor_tensor(
                        v[:], py[:], rep[:, 3 * k + 1 : 3 * k + 2], u[:], Alu.mult, Alu.add
                    )
                    nc.vector.scalar_tensor_tensor(
                        s[:], pz[:], rep[:, 3 * k + 2 : 3 * k + 3], v[:], Alu.mult, Alu.add
                    )
                    s_tiles.append(s)

                # ---------- phase 2: m = max_k s_k ----------
                m01 = scr.tile([P, F], f32, tag="m01")
                m23 = scr.tile([P, F], f32, tag="m23")
                m = sb.tile([P, F], f32, tag="m")
                nc.vector.tensor_tensor(m01[:], s_tiles[0][:], s_tiles[1][:], Alu.max)
                nc.vector.tensor_tensor(m23[:], s_tiles[2][:], s_tiles[3][:], Alu.max)
                nc.vector.tensor_tensor(m01[:], m01[:], s_tiles[4][:], Alu.max)
                nc.vector.tensor_tensor(m[:], m01[:], m23[:], Alu.max)

                # ---------- phase 3: masks, counts, sums ----------
                acc_d = sb.tile([P, 16], f32, tag="acc_d")   # cnt0..3, S0xyz..S3xyz
                junk_a = scr.tile([P, F], f32, tag="junk_a")
                for k in range(4):
                    mask = scr.tile([P, F], f32, tag=f"mask{k}")
                    nc.vector.tensor_tensor(mask[:], s_tiles[k][:], m[:], Alu.is_equal)
                    # counts via ACT fused row-accumulate
                    nc.scalar.activation(
                        junk_a[:], mask[:], Act.Identity,
                        accum_out=acc_d[:, k : k + 1],
                    )
                    # products for all 3 channels in one strided TT
                    prod3 = scr.tile([P, 3 * F], f32, tag=f"prod{k}")
                    nc.vector.tensor_tensor(
                        prod3[:].rearrange("p (d f) -> p f d", f=F),
                        mask[:].rearrange("p (f o) -> p f o", o=1).broadcast_to((P, F, 3)),
                        pint[:].rearrange("p (f d) -> p f d", d=3),
                        Alu.mult,
                    )
                    for d in range(3):
                        nc.scalar.activation(
                            junk_a[:], prod3[:, d * F : (d + 1) * F], Act.Identity,
                            accum_out=acc_d[:, 4 + 3 * k + d : 5 + 3 * k + d],
                        )

                # ---------- tail: totals -> new centers ----------
                tot = ps.tile([1, 16], f32, tag="tot")
                nc.tensor.matmul(tot[:], ones_col[:], acc_d[:], start=True, stop=True)
                tots = sm.tile([1, 16], f32, tag="tots")
                nc.vector.tensor_copy(tots[:], tot[:])

                cnts = sm.tile([1, 5], f32, tag="cnts")
                csum = sm.tile([1, 1], f32, tag="csum")
                nc.vector.tensor_copy(cnts[0:1, 0:4], tots[0:1, 0:4])
                nc.vector.tensor_reduce(csum[:], tots[0:1, 0:4], mybir.AxisListType.X, Alu.add)
                nc.vector.tensor_scalar(cnts[0:1, 4:5], csum[:], -1.0, float(N), Alu.mult, Alu.add)

                S15 = sm.tile([1, 15], f32, tag="S15")
                s4p = sm.tile([1, 3], f32, tag="s4p")
                nc.vector.tensor_copy(S15[0:1, 0:12], tots[0:1, 4:16])
                # sum over k of S_kd: view cols 4..16 as [d(stride1,3), k(stride3,4)], reduce X
                nc.vector.tensor_reduce(
                    s4p[:], tots[0:1, 4:16].rearrange("p (k d) -> p d k", d=3),
                    mybir.AxisListType.X, Alu.add,
                )
                nc.vector.tensor_tensor(S15[0:1, 12:15], tot3[:], s4p[:], Alu.subtract)

                recip = sm.tile([1, 5], f32, tag="recip")
                nc.vector.reciprocal(recip[:], cnts[:])
                recip15 = sm.tile([1, 15], f32, tag="recip15")
                for d in range(3):
                    nc.scalar.copy(recip15[0:1, d:15:3].rearrange("p (a b) -> p (b a)", b=1), recip[:])

                cb = sm.tile([1, 20], f32, tag="cb")
                nc.vector.tensor_tensor(cb[0:1, 0:15], S15[:], recip15[:], Alu.mult)

                sq = sm.tile([1, 15], f32, tag="sq")
                c2 = sm.tile([1, 5], f32, tag="c2")
                nc.vector.tensor_tensor(sq[:], cb[0:1, 0:15], cb[0:1, 0:15], Alu.mult)
                nc.vector.tensor_reduce(
                    c2[:], sq[:].rearrange("p (k d) -> p k d", d=3),
                    mybir.AxisListType.X, Alu.add,
                )
                nc.vector.tensor_scalar(cb[0:1, 15:20], c2[:], -0.5, 2.0, Alu.mult, Alu.add)

                # store trajectories (ScalarE, off critical path)
                nc.scalar.copy(counts_st[0:1, 5 * (t - 1) : 5 * t], cnts[:])
                nc.scalar.copy(cent_st[0:1, 15 * t : 15 * (t + 1)], cb[0:1, 0:15])

                # broadcast for next iteration
                rep_ps = ps.tile([P, 20], f32, tag="repps")
                nc.tensor.matmul(rep_ps[:], ones_row[:], cb[:], start=True, stop=True)
                rep = sb.tile([P, 20], f32, tag="rep")
                nc.vector.tensor_copy(rep[:], rep_ps[:])
                rep_g = sb.tile([P, 20], f32, tag="rep_g")
                nc.gpsimd.tensor_copy(rep_g[:], rep[:])

            nc.sync.dma_start(out=outv[0:1, 0:500], in_=counts_st[:])
            nc.sync.dma_start(out=outv[0:1, 500:OUT_LEN], in_=cent_st[:])
    nc.compile()
    return nc


def _get_nc():
    if "nc" not in _CACHE:
        _CACHE["nc"] = _build_nc()
    return _CACHE["nc"]


def _host_finalize(counts_all, cent_all):
    """counts_all [B,100,5], cent_all [B,101,15] -> [B,K,K,4] per reference."""
    B = counts_all.shape[0]
    prev = cent_all[:, :-1, :]   # centers entering iter t (t=1..100)
    new = cent_all[:, 1:, :]     # new_centers at iter t
    with np.errstate(invalid="ignore"):
        ok = np.abs(prev - new) <= np.float32(ATOL) + np.float32(RTOL) * np.abs(new)
    conv_t = np.all(ok, axis=(0, 2))          # [100] global allclose per iter
    idx = np.nonzero(conv_t)[0]
    T = int(idx[0]) + 1 if len(idx) else ITERS + 1
    L = min(T, ITERS)
    centers = cent_all[:, T - 1].reshape(B, K, 3)
    percentages = counts_all[:, L - 1] / np.float32(N)
    centers = np.clip(centers, 0.0, 1.0)
    percentages = np.clip(percentages, 0.0, 1.0)
    color_info = np.concatenate([centers, percentages[..., None]], axis=2).astype(np.float32)
    color_info = np.nan_to_num(color_info, nan=0.0, posinf=1.0, neginf=0.0)
    sort_idx = np.argsort(-color_info[:, :, 3], axis=1, kind="stable")
    return color_info[sort_idx]


def _make_inputs(x, init_idx):
    B = x.shape[0]
    x = np.ascontiguousarray(np.asarray(x, dtype=np.float32))
    init_idx = np.asarray(init_idx).astype(np.int64)
    hh, ww = init_idx // 224, init_idx % 224
    in_maps = []
    for b in range(B):
        c0 = (x[b, :, hh, ww] + np.float32(1e-8)).astype(np.float32)  # [5,3]
        cb0 = np.zeros((1, 20), np.float32)
        cb0[0, :15] = c0.reshape(15)
        c2 = (c0 * c0).sum(axis=1, dtype=np.float32)
        cb0[0, 15:20] = np.float32(2.0) - np.float32(0.5) * c2
        in_maps.append({"xp": x[b].reshape(3, N), "cbin": cb0})
    return in_maps


def kernel(x, init_idx):
    from concourse.bass_utils import run_bass_kernel_spmd

    nc = _get_nc()
    in_maps = _make_inputs(x, init_idx)
    res = run_bass_kernel_spmd(nc, in_maps, list(range(8)))
    outs = [np.asarray(r["outv"]).reshape(OUT_LEN) for r in res.results]
    counts_all = np.stack([o[0:500].reshape(100, 5) for o in outs])
    cent_all = np.stack([o[500:OUT_LEN].reshape(101, 15) for o in outs])
    return _host_finalize(counts_all, cent_all)

